# revision 14
# baseline (speedup 1.0000x reference)
"""Trainium2 Bass kernel for the MNIST-style CNN (conv3x3 -> conv3x3 ->
maxpool2x2 -> fc 9216->128 -> fc 128->10), data-parallel over 8 NeuronCores.

Layouts (per core, BC=512 images):
  conv1: Toeplitz-over-y matmul. K = 84 = (3 dx x 28 yi) input rows,
         M = 128 = (4 yo x 32 c) outputs, N = (16 img x 26 xo).
         rhs = x3[dx*28+yi, img, xc] = x[img, yi, xc+dx] (host-prepped).
  h1:    stored in 12 overlapping row-groups per 32-image chunk:
         group t = [128 p=(4 yi_local x 32 c), (32 img, 26 xi)], yi = 2t+yi_local.
  conv2: K = 96 = (3 dy x 32 c), accumulate over 3 dx into PSUM.
         yo=2t uses partitions 0:96 of group t, yo=2t+1 uses 32:128.
         Two 16-image groups run concurrently via col-tiling (M=64 each,
         PSUM partitions 0:64 / 64:128).
  pool:  3 DVE max ops from PSUM, then 2 ACT relu+bias ops scatter into
         fc1 layout: pooled[p=(yp%2)*64+c, (img128, yp//2, xo12)].
  fc1:   72 accumulating matmuls K=128=(yp-parity x 64 c), M=128 j, N=128 img.
  fc2:   single matmul K=128, M=10, N=512. Output [10, 512] transposed on host.
"""

import sys

if "/opt/trn_rl_repo" not in sys.path:
    sys.path.insert(0, "/opt/trn_rl_repo")

import os

import numpy as np

NCORES = 8
_BUILT = {}
# matmul operand dtype: "float32" (exact, dual-pass), "float32r" (relaxed
# single-pass), or "bfloat16"
MM_DT = os.environ.get("KERNEL_MM_DT", "float32r")


def _host_prep(x, w1, b1, w2, b2, fc1_w, fc1_b, fc2_w, fc2_b):
    B = x.shape[0]
    xs = np.ascontiguousarray(x[:, 0], np.float32)          # [B, 28, 28]
    xT = np.ascontiguousarray(xs.transpose(1, 0, 2))        # [28 yi, B, 28 xc]
    x3 = np.zeros((84, B, 28), np.float32)
    for dx in range(3):
        x3[dx * 28:(dx + 1) * 28, :, :28 - dx] = xT[:, :, dx:]

    # conv1 Toeplitz weights: W1p[dx*28+yi, g, yl*32+c] = w1[c, 0, yi-(4g+yl), dx]
    W1p = np.zeros((84, 7, 128), np.float32)
    for dx in range(3):
        for r in range(3):
            for g in range(7):
                for yl in range(4):
                    yi = 4 * g + yl + r
                    if yi < 28:
                        W1p[dx * 28 + yi, g, yl * 32:(yl + 1) * 32] = w1[:, 0, r, dx]
    b1t = np.ascontiguousarray(np.tile(b1, 4)[:, None], np.float32)     # [128,1]

    # conv2 weights: w2k[dy*32+c, dx, co] = w2[co, c, dy, dx]. Two zero-padded
    # K=128 variants: even yo contracts h1 rows 0:96 (yi_local 0..2), odd yo
    # rows 32:128 (yi_local 1..3); the unused 32 rows get zero weights.
    w2k = np.ascontiguousarray(
        w2.transpose(2, 1, 3, 0).reshape(96, 3, 64), np.float32)
    w2lo = np.zeros((128, 3, 64), np.float32)
    w2hi = np.zeros((128, 3, 64), np.float32)
    w2lo[0:96] = w2k
    w2hi[32:128] = w2k
    w2cat = np.concatenate([w2lo, w2hi], axis=2)  # [128, 3, 128]
    b2t = np.ascontiguousarray(np.tile(b2, 2)[:, None], np.float32)     # [128,1]

    # fc1: fc1wp[par*64+c, t2*12+xo, j] = fc1_w[j, c*144 + (2*t2+par)*12 + xo]
    fw = fc1_w.reshape(128, 64, 12, 12).transpose(1, 2, 3, 0)  # [c, yp, xo, j]
    fc1wp = np.ascontiguousarray(
        np.stack([fw[:, 0::2], fw[:, 1::2]]).reshape(128, 72, 128), np.float32)
    fc1bt = np.ascontiguousarray(fc1_b[:, None], np.float32)
    fc2wt = np.ascontiguousarray(fc2_w.T, np.float32)                   # [128,10]
    fc2bt = np.ascontiguousarray(fc2_b[:, None], np.float32)            # [10,1]

    weights = dict(w1p=W1p, b1t=b1t, w2cat=w2cat, b2t=b2t,
                   fc1wp=fc1wp, fc1bt=fc1bt, fc2wt=fc2wt, fc2bt=fc2bt)
    if MM_DT == "bfloat16":
        import ml_dtypes
        bf = ml_dtypes.bfloat16
        x3 = x3.astype(bf)
        for k in ("w1p", "w2cat", "fc1wp", "fc2wt"):
            weights[k] = weights[k].astype(bf)
    return x3, weights


def _build(bc):
    from concourse import bacc, tile
    import concourse.mybir as mybir

    from concourse.alu_op_type import AluOpType as Alu

    f32 = mybir.dt.float32
    mmdt = getattr(mybir.dt, MM_DT)
    Relu = mybir.ActivationFunctionType.Relu

    nc = bacc.Bacc("TRN2", target_bir_lowering=False, debug=False)

    x3_d = nc.dram_tensor("x3", [84, bc, 28], mmdt, kind="ExternalInput")
    w1_d = nc.dram_tensor("w1p", [84, 7, 128], mmdt, kind="ExternalInput")
    b1_d = nc.dram_tensor("b1t", [128, 1], f32, kind="ExternalInput")
    w2_d = nc.dram_tensor("w2cat", [128, 3, 128], mmdt, kind="ExternalInput")
    b2_d = nc.dram_tensor("b2t", [128, 1], f32, kind="ExternalInput")
    f1w_d = nc.dram_tensor("fc1wp", [128, 72, 128], mmdt, kind="ExternalInput")
    f1b_d = nc.dram_tensor("fc1bt", [128, 1], f32, kind="ExternalInput")
    f2w_d = nc.dram_tensor("fc2wt", [128, 10], mmdt, kind="ExternalInput")
    f2b_d = nc.dram_tensor("fc2bt", [10, 1], f32, kind="ExternalInput")
    out_d = nc.dram_tensor("out", [10, bc], f32, kind="ExternalOutput")

    n_sc = bc // 128            # fc1 superchunks of 128 images

    with tile.TileContext(nc) as tc:
        with (
            tc.tile_pool(name="wpool", bufs=1) as wp,
            tc.tile_pool(name="xpool", bufs=4) as xp,
            tc.tile_pool(name="h1pool", bufs=24) as h1p,
            tc.tile_pool(name="tmp", bufs=6) as tp,
            tc.tile_pool(name="pooled", bufs=1) as plp,
            tc.tile_pool(name="fc1h", bufs=1) as fhp,
            tc.tile_pool(name="outp", bufs=1) as op_,
            tc.tile_pool(name="ps1", bufs=3, space="PSUM") as ps1,
            tc.tile_pool(name="ps2", bufs=4, space="PSUM") as ps2,
            tc.tile_pool(name="psf", bufs=1, space="PSUM") as psf,
        ):
            w1_sb = wp.tile([84, 7, 128], mmdt)
            nc.sync.dma_start(w1_sb[:], w1_d[:])
            w2_sb = wp.tile([128, 3, 128], mmdt)
            nc.sync.dma_start(w2_sb[:], w2_d[:])
            f1w_sb = wp.tile([128, 72, 128], mmdt)
            nc.sync.dma_start(f1w_sb[:], f1w_d[:])
            f2w_sb = wp.tile([128, 10], mmdt)
            nc.sync.dma_start(f2w_sb[:], f2w_d[:])
            b1_sb = wp.tile([128, 1], f32)
            nc.sync.dma_start(b1_sb[:], b1_d[:])
            b2_sb = wp.tile([128, 1], f32)
            nc.sync.dma_start(b2_sb[:], b2_d[:])
            f1b_sb = wp.tile([128, 1], f32)
            nc.sync.dma_start(f1b_sb[:], f1b_d[:])
            f2b_sb = wp.tile([10, 1], f32)
            nc.sync.dma_start(f2b_sb[:], f2b_d[:])

            fc1h = fhp.tile([128, n_sc, 128], mmdt)

            for sc in range(n_sc):
                pooled = plp.tile([128, 128, 6, 12], mmdt, tag="pooled")
                for pci in range(4):
                    pc = sc * 4 + pci
                    h1t = [h1p.tile([128, 32, 26], mmdt, tag="h1",
                                    name=f"h1_{pc}_{j}")
                           for j in range(12)]
                    for half in range(2):
                        i0 = pc * 32 + half * 16
                        hs = slice(half * 16, half * 16 + 16)
                        x3t = xp.tile([84, 16, 28], mmdt, tag="x3")
                        nc.sync.dma_start(x3t[:], x3_d[:, i0:i0 + 16, :])
                        for g in range(7):
                            p = ps1.tile([128, 16, 26], f32, tag="c1ps")
                            nc.tensor.matmul(p[:], w1_sb[:, g, :],
                                             x3t[:, :, 0:26],
                                             start=True, stop=True)
                            # scatter psum rows (yi = 4g..4g+3) into the
                            # overlapping h1 row-groups, fused relu+bias
                            if g <= 5:
                                nc.scalar.activation(
                                    h1t[2 * g][:, hs, :], p[:], Relu,
                                    bias=b1_sb[:])
                            if g >= 1:
                                nc.scalar.activation(
                                    h1t[2 * g - 1][64:128, hs, :], p[0:64],
                                    Relu, bias=b1_sb[0:64])
                            if g <= 5:
                                nc.scalar.activation(
                                    h1t[2 * g + 1][0:64, hs, :], p[64:128],
                                    Relu, bias=b1_sb[64:128])

                    for t in range(12):
                        par = (t % 2) * 64
                        t2 = t // 2
                        i0 = pci * 32
                        h2t = tp.tile([128, 32, 12, 2], f32, tag="h2",
                                      name=f"h2_{pc}_{t}")
                        for half in range(2):
                            hsl = slice(half * 16, half * 16 + 16)
                            pt = ps2.tile([128, 16, 12, 2], f32, tag="c2ps",
                                          name=f"c2_{pc}_{t}_{half}")
                            for dx in range(3):
                                nc.tensor.matmul(
                                    pt[:], w2_sb[:, dx, :],
                                    h1t[t][:, hsl, dx:dx + 24],
                                    start=(dx == 0), stop=(dx == 2))
                            # drain psum with fused relu+bias
                            if (t + half) % 3 == 0:
                                nc.vector.tensor_scalar(
                                    h2t[:, hsl, :, :], pt[:], b2_sb[:], 0.0,
                                    op0=Alu.add, op1=Alu.max)
                            else:
                                nc.scalar.activation(h2t[:, hsl, :, :], pt[:],
                                                     Relu, bias=b2_sb[:])
                        # pool: xo-pair maxes per parity half (same-base
                        # inputs), then cross-parity max into pooled
                        me = tp.tile([64, 32, 12], f32, tag="me",
                                     name=f"me_{pc}_{t}")
                        mo = tp.tile([64, 32, 12], f32, tag="mo",
                                     name=f"mo_{pc}_{t}")
                        nc.vector.tensor_max(me[:], h2t[0:64, :, :, 0],
                                             h2t[0:64, :, :, 1])
                        nc.vector.tensor_max(mo[:], h2t[64:128, :, :, 0],
                                             h2t[64:128, :, :, 1])
                        nc.vector.tensor_max(
                            pooled[par:par + 64, i0:i0 + 32, t2, :],
                            me[:], mo[:])
                fps = psf.tile([128, 128], f32, tag="fps", padded_shape=[128, 512])
                for r in range(72):
                    nc.tensor.matmul(fps[:], f1w_sb[:, r, :],
                                     pooled[:, :, r // 12, r % 12],
                                     start=(r == 0), stop=(r == 71))
                nc.scalar.activation(fc1h[:, sc, :], fps[:], Relu,
                                     bias=f1b_sb[:])

            f2ps = psf.tile([10, bc], f32, tag="fps")
            nc.tensor.matmul(f2ps[:], f2w_sb[:], fc1h[:],
                             start=True, stop=True)
            outsb = op_.tile([10, bc], f32)
            nc.vector.tensor_scalar_add(outsb[:], f2ps[:], f2b_sb[:])
            nc.sync.dma_start(out_d[:], outsb[:])

    nc.compile()
    return nc


def _get_nc(bc):
    key = (bc, MM_DT)
    if key not in _BUILT:
        _BUILT[key] = _build(bc)
    return _BUILT[key]


def run_cores(x, w1, b1, w2, b2, fc1_w, fc1_b, fc2_w, fc2_b,
              trace=False, ncores=NCORES):
    from concourse.bass_utils import run_bass_kernel_spmd

    B = x.shape[0]
    bc = B // ncores
    nc = _get_nc(bc)
    x3, weights = _host_prep(x, w1, b1, w2, b2, fc1_w, fc1_b,
                             fc2_w, fc2_b)
    in_maps = []
    for c in range(ncores):
        m = dict(weights)
        m["x3"] = np.ascontiguousarray(x3[:, c * bc:(c + 1) * bc, :])
        in_maps.append(m)
    res = run_bass_kernel_spmd(nc, in_maps, core_ids=list(range(ncores)),
                               trace=trace)
    out = np.empty((B, 10), np.float32)
    for c in range(ncores):
        out[c * bc:(c + 1) * bc] = res.results[c]["out"].T
    return out, res


def kernel(x, w1, b1, w2, b2, fc1_w, fc1_b, fc2_w, fc2_b):
    out, _ = run_cores(x, w1, b1, w2, b2, fc1_w, fc1_b, fc2_w, fc2_b)
    return out


# revision 15
# speedup vs baseline: 1.0572x; 1.0572x over previous
"""Trainium2 Bass kernel for the MNIST-style CNN (conv3x3 -> conv3x3 ->
maxpool2x2 -> fc 9216->128 -> fc 128->10), data-parallel over 8 NeuronCores.

Layouts (per core, BC=512 images):
  conv1: Toeplitz-over-y matmul. K = 84 = (3 dx x 28 yi) input rows,
         M = 128 = (4 yo x 32 c) outputs, N = (16 img x 26 xo).
         rhs = x3[dx*28+yi, img, xc] = x[img, yi, xc+dx] (host-prepped).
  h1:    stored in 12 overlapping row-groups per 32-image chunk:
         group t = [128 p=(4 yi_local x 32 c), (32 img, 26 xi)], yi = 2t+yi_local.
  conv2: K = 96 = (3 dy x 32 c), accumulate over 3 dx into PSUM.
         yo=2t uses partitions 0:96 of group t, yo=2t+1 uses 32:128.
         Two 16-image groups run concurrently via col-tiling (M=64 each,
         PSUM partitions 0:64 / 64:128).
  pool:  3 DVE max ops from PSUM, then 2 ACT relu+bias ops scatter into
         fc1 layout: pooled[p=(yp%2)*64+c, (img128, yp//2, xo12)].
  fc1:   72 accumulating matmuls K=128=(yp-parity x 64 c), M=128 j, N=128 img.
  fc2:   single matmul K=128, M=10, N=512. Output [10, 512] transposed on host.
"""

import sys

if "/opt/trn_rl_repo" not in sys.path:
    sys.path.insert(0, "/opt/trn_rl_repo")

import os

import numpy as np

NCORES = 8
_BUILT = {}
# matmul operand dtype: "float32" (exact, dual-pass), "float32r" (relaxed
# single-pass), or "bfloat16"
MM_DT = os.environ.get("KERNEL_MM_DT", "float32r")


def _host_prep(x, w1, b1, w2, b2, fc1_w, fc1_b, fc2_w, fc2_b):
    B = x.shape[0]
    xs = np.ascontiguousarray(x[:, 0], np.float32)          # [B, 28, 28]
    xT = np.ascontiguousarray(xs.transpose(1, 0, 2))        # [28 yi, B, 28 xc]
    x3 = np.zeros((84, B, 28), np.float32)
    for dx in range(3):
        x3[dx * 28:(dx + 1) * 28, :, :28 - dx] = xT[:, :, dx:]

    # conv1 Toeplitz weights: W1p[dx*28+yi, g, yl*32+c] = w1[c, 0, yi-(4g+yl), dx]
    W1p = np.zeros((84, 7, 128), np.float32)
    for dx in range(3):
        for r in range(3):
            for g in range(7):
                for yl in range(4):
                    yi = 4 * g + yl + r
                    if yi < 28:
                        W1p[dx * 28 + yi, g, yl * 32:(yl + 1) * 32] = w1[:, 0, r, dx]
    b1t = np.ascontiguousarray(np.tile(b1, 4)[:, None], np.float32)     # [128,1]

    # conv2 weights: w2k[dy*32+c, dx, co] = w2[co, c, dy, dx]. Two zero-padded
    # K=128 variants: even yo contracts h1 rows 0:96 (yi_local 0..2), odd yo
    # rows 32:128 (yi_local 1..3); the unused 32 rows get zero weights.
    w2k = np.ascontiguousarray(
        w2.transpose(2, 1, 3, 0).reshape(96, 3, 64), np.float32)
    w2lo = np.zeros((128, 3, 64), np.float32)
    w2hi = np.zeros((128, 3, 64), np.float32)
    w2lo[0:96] = w2k
    w2hi[32:128] = w2k
    w2cat = np.concatenate([w2lo, w2hi], axis=2)  # [128, 3, 128]
    b2t = np.ascontiguousarray(np.tile(b2, 2)[:, None], np.float32)     # [128,1]

    # fc1: fc1wp[par*64+c, t2*12+xo, j] = fc1_w[j, c*144 + (2*t2+par)*12 + xo]
    fw = fc1_w.reshape(128, 64, 12, 12).transpose(1, 2, 3, 0)  # [c, yp, xo, j]
    fc1wp = np.ascontiguousarray(
        np.stack([fw[:, 0::2], fw[:, 1::2]]).reshape(128, 72, 128), np.float32)
    fc1bt = np.ascontiguousarray(fc1_b[:, None], np.float32)
    fc2wt = np.ascontiguousarray(fc2_w.T, np.float32)                   # [128,10]
    fc2bt = np.ascontiguousarray(fc2_b[:, None], np.float32)            # [10,1]

    weights = dict(w1p=W1p, b1t=b1t, w2cat=w2cat, b2t=b2t,
                   fc1wp=fc1wp, fc1bt=fc1bt, fc2wt=fc2wt, fc2bt=fc2bt)
    if MM_DT == "bfloat16":
        import ml_dtypes
        bf = ml_dtypes.bfloat16
        x3 = x3.astype(bf)
        for k in ("w1p", "w2cat", "fc1wp", "fc2wt"):
            weights[k] = weights[k].astype(bf)
    return x3, weights


def _build(bc):
    from concourse import bacc, tile
    import concourse.mybir as mybir

    from concourse.alu_op_type import AluOpType as Alu

    f32 = mybir.dt.float32
    mmdt = getattr(mybir.dt, MM_DT)
    Relu = mybir.ActivationFunctionType.Relu

    nc = bacc.Bacc("TRN2", target_bir_lowering=False, debug=False)

    x3_d = nc.dram_tensor("x3", [84, bc, 28], mmdt, kind="ExternalInput")
    w1_d = nc.dram_tensor("w1p", [84, 7, 128], mmdt, kind="ExternalInput")
    b1_d = nc.dram_tensor("b1t", [128, 1], f32, kind="ExternalInput")
    w2_d = nc.dram_tensor("w2cat", [128, 3, 128], mmdt, kind="ExternalInput")
    b2_d = nc.dram_tensor("b2t", [128, 1], f32, kind="ExternalInput")
    f1w_d = nc.dram_tensor("fc1wp", [128, 72, 128], mmdt, kind="ExternalInput")
    f1b_d = nc.dram_tensor("fc1bt", [128, 1], f32, kind="ExternalInput")
    f2w_d = nc.dram_tensor("fc2wt", [128, 10], mmdt, kind="ExternalInput")
    f2b_d = nc.dram_tensor("fc2bt", [10, 1], f32, kind="ExternalInput")
    out_d = nc.dram_tensor("out", [10, bc], f32, kind="ExternalOutput")

    n_sc = bc // 128            # fc1 superchunks of 128 images

    with tile.TileContext(nc) as tc:
        with (
            tc.tile_pool(name="wpool", bufs=1) as wp,
            tc.tile_pool(name="xpool", bufs=4) as xp,
            tc.tile_pool(name="h1pool", bufs=24) as h1p,
            tc.tile_pool(name="tmp", bufs=6) as tp,
            tc.tile_pool(name="pooled", bufs=1) as plp,
            tc.tile_pool(name="fc1h", bufs=1) as fhp,
            tc.tile_pool(name="outp", bufs=1) as op_,
            tc.tile_pool(name="ps1", bufs=3, space="PSUM") as ps1,
            tc.tile_pool(name="ps2", bufs=4, space="PSUM") as ps2,
            tc.tile_pool(name="psf", bufs=1, space="PSUM") as psf,
        ):
            w1_sb = wp.tile([84, 7, 128], mmdt)
            nc.sync.dma_start(w1_sb[:], w1_d[:])
            w2_sb = wp.tile([128, 3, 128], mmdt)
            nc.sync.dma_start(w2_sb[:], w2_d[:])
            f1w_sb = wp.tile([128, 72, 128], mmdt)
            nc.sync.dma_start(f1w_sb[:], f1w_d[:])
            f2w_sb = wp.tile([128, 10], mmdt)
            nc.sync.dma_start(f2w_sb[:], f2w_d[:])
            b1_sb = wp.tile([128, 1], f32)
            nc.sync.dma_start(b1_sb[:], b1_d[:])
            b2_sb = wp.tile([128, 1], f32)
            nc.sync.dma_start(b2_sb[:], b2_d[:])
            f1b_sb = wp.tile([128, 1], f32)
            nc.sync.dma_start(f1b_sb[:], f1b_d[:])
            f2b_sb = wp.tile([10, 1], f32)
            nc.sync.dma_start(f2b_sb[:], f2b_d[:])

            fc1h = fhp.tile([128, n_sc, 128], mmdt)

            for sc in range(n_sc):
                pooled = plp.tile([128, 128, 6, 12], mmdt, tag="pooled")
                for pci in range(4):
                    pc = sc * 4 + pci
                    h1t = [h1p.tile([128, 32, 26], mmdt, tag="h1",
                                    name=f"h1_{pc}_{j}")
                           for j in range(12)]
                    for half in range(2):
                        i0 = pc * 32 + half * 16
                        hs = slice(half * 16, half * 16 + 16)
                        x3t = xp.tile([84, 16, 28], mmdt, tag="x3")
                        nc.sync.dma_start(x3t[:], x3_d[:, i0:i0 + 16, :])
                        for g in range(7):
                            p = ps1.tile([128, 16, 26], f32, tag="c1ps")
                            nc.tensor.matmul(p[:], w1_sb[:, g, :],
                                             x3t[:, :, 0:26],
                                             start=True, stop=True)
                            # scatter psum rows (yi = 4g..4g+3) into the
                            # overlapping h1 row-groups, fused relu+bias
                            if g <= 5:
                                nc.scalar.activation(
                                    h1t[2 * g][:, hs, :], p[:], Relu,
                                    bias=b1_sb[:])
                            if g >= 1:
                                nc.scalar.activation(
                                    h1t[2 * g - 1][64:128, hs, :], p[0:64],
                                    Relu, bias=b1_sb[0:64])
                            if g <= 5:
                                nc.scalar.activation(
                                    h1t[2 * g + 1][0:64, hs, :], p[64:128],
                                    Relu, bias=b1_sb[64:128])

                    for t in range(12):
                        par = (t % 2) * 64
                        t2 = t // 2
                        i0 = pci * 32
                        h2t = tp.tile([128, 32, 12, 2], mmdt, tag="h2",
                                      name=f"h2_{pc}_{t}")
                        for half in range(2):
                            hsl = slice(half * 16, half * 16 + 16)
                            pt = ps2.tile([128, 16, 12, 2], f32, tag="c2ps",
                                          name=f"c2_{pc}_{t}_{half}")
                            for dx in range(3):
                                nc.tensor.matmul(
                                    pt[:], w2_sb[:, dx, :],
                                    h1t[t][:, hsl, dx:dx + 24],
                                    start=(dx == 0), stop=(dx == 2))
                            # drain psum with fused relu+bias
                            if (t + half) % 3 == 0:
                                nc.vector.tensor_scalar(
                                    h2t[:, hsl, :, :], pt[:], b2_sb[:], 0.0,
                                    op0=Alu.add, op1=Alu.max)
                            else:
                                nc.scalar.activation(h2t[:, hsl, :, :], pt[:],
                                                     Relu, bias=b2_sb[:])
                        # pool: xo-pair maxes per parity half (same-base
                        # inputs), then cross-parity max into pooled
                        me = tp.tile([64, 32, 12], mmdt, tag="me",
                                     name=f"me_{pc}_{t}")
                        mo = tp.tile([64, 32, 12], mmdt, tag="mo",
                                     name=f"mo_{pc}_{t}")
                        nc.vector.tensor_max(me[:], h2t[0:64, :, :, 0],
                                             h2t[0:64, :, :, 1])
                        nc.vector.tensor_max(mo[:], h2t[64:128, :, :, 0],
                                             h2t[64:128, :, :, 1])
                        nc.vector.tensor_max(
                            pooled[par:par + 64, i0:i0 + 32, t2, :],
                            me[:], mo[:])
                fps = psf.tile([128, 128], f32, tag="fps", padded_shape=[128, 512])
                for r in range(72):
                    nc.tensor.matmul(fps[:], f1w_sb[:, r, :],
                                     pooled[:, :, r // 12, r % 12],
                                     start=(r == 0), stop=(r == 71))
                nc.scalar.activation(fc1h[:, sc, :], fps[:], Relu,
                                     bias=f1b_sb[:])

            f2ps = psf.tile([10, bc], f32, tag="fps")
            nc.tensor.matmul(f2ps[:], f2w_sb[:], fc1h[:],
                             start=True, stop=True)
            outsb = op_.tile([10, bc], f32)
            nc.vector.tensor_scalar_add(outsb[:], f2ps[:], f2b_sb[:])
            nc.sync.dma_start(out_d[:], outsb[:])

    nc.compile()
    return nc


def _get_nc(bc):
    key = (bc, MM_DT)
    if key not in _BUILT:
        _BUILT[key] = _build(bc)
    return _BUILT[key]


def run_cores(x, w1, b1, w2, b2, fc1_w, fc1_b, fc2_w, fc2_b,
              trace=False, ncores=NCORES):
    from concourse.bass_utils import run_bass_kernel_spmd

    B = x.shape[0]
    bc = B // ncores
    nc = _get_nc(bc)
    x3, weights = _host_prep(x, w1, b1, w2, b2, fc1_w, fc1_b,
                             fc2_w, fc2_b)
    in_maps = []
    for c in range(ncores):
        m = dict(weights)
        m["x3"] = np.ascontiguousarray(x3[:, c * bc:(c + 1) * bc, :])
        in_maps.append(m)
    res = run_bass_kernel_spmd(nc, in_maps, core_ids=list(range(ncores)),
                               trace=trace)
    out = np.empty((B, 10), np.float32)
    for c in range(ncores):
        out[c * bc:(c + 1) * bc] = res.results[c]["out"].T
    return out, res


def kernel(x, w1, b1, w2, b2, fc1_w, fc1_b, fc2_w, fc2_b):
    out, _ = run_cores(x, w1, b1, w2, b2, fc1_w, fc1_b, fc2_w, fc2_b)
    return out


# revision 16
# speedup vs baseline: 1.1401x; 1.0784x over previous
"""Trainium2 Bass kernel for the MNIST-style CNN (conv3x3 -> conv3x3 ->
maxpool2x2 -> fc 9216->128 -> fc 128->10), data-parallel over 8 NeuronCores.

Layouts (per core, BC=512 images):
  conv1: Toeplitz-over-y matmul. K = 84 = (3 dx x 28 yi) input rows,
         M = 128 = (4 yo x 32 c) outputs, N = (16 img x 26 xo).
         rhs = x3[dx*28+yi, img, xc] = x[img, yi, xc+dx] (host-prepped).
  h1:    stored in 12 overlapping row-groups per 32-image chunk:
         group t = [128 p=(4 yi_local x 32 c), (32 img, 26 xi)], yi = 2t+yi_local.
  conv2: K = 96 = (3 dy x 32 c), accumulate over 3 dx into PSUM.
         yo=2t uses partitions 0:96 of group t, yo=2t+1 uses 32:128.
         Two 16-image groups run concurrently via col-tiling (M=64 each,
         PSUM partitions 0:64 / 64:128).
  pool:  3 DVE max ops from PSUM, then 2 ACT relu+bias ops scatter into
         fc1 layout: pooled[p=(yp%2)*64+c, (img128, yp//2, xo12)].
  fc1:   72 accumulating matmuls K=128=(yp-parity x 64 c), M=128 j, N=128 img.
  fc2:   single matmul K=128, M=10, N=512. Output [10, 512] transposed on host.
"""

import sys

if "/opt/trn_rl_repo" not in sys.path:
    sys.path.insert(0, "/opt/trn_rl_repo")

import os

import numpy as np

NCORES = 8
_BUILT = {}
# matmul operand dtype: "float32" (exact, dual-pass), "float32r" (relaxed
# single-pass), or "bfloat16"
MM_DT = os.environ.get("KERNEL_MM_DT", "float32r")


def _host_prep(x, w1, b1, w2, b2, fc1_w, fc1_b, fc2_w, fc2_b):
    B = x.shape[0]
    xs = np.ascontiguousarray(x[:, 0], np.float32)          # [B, 28, 28]
    xT = np.ascontiguousarray(xs.transpose(1, 0, 2))        # [28 yi, B, 28 xc]
    x3 = np.zeros((84, B, 28), np.float32)
    for dx in range(3):
        x3[dx * 28:(dx + 1) * 28, :, :28 - dx] = xT[:, :, dx:]

    # conv1 Toeplitz weights: W1p[dx*28+yi, g, yl*32+c] = w1[c, 0, yi-(4g+yl), dx]
    W1p = np.zeros((84, 12, 128), np.float32)
    for dx in range(3):
        for r in range(3):
            for g in range(12):
                for yl in range(4):
                    yi = 2 * g + yl + r
                    if yi < 28:
                        W1p[dx * 28 + yi, g, yl * 32:(yl + 1) * 32] = w1[:, 0, r, dx]
    b1t = np.ascontiguousarray(np.tile(b1, 4)[:, None], np.float32)     # [128,1]

    # conv2 weights: w2k[dy*32+c, dx, co] = w2[co, c, dy, dx]. Two zero-padded
    # K=128 variants: even yo contracts h1 rows 0:96 (yi_local 0..2), odd yo
    # rows 32:128 (yi_local 1..3); the unused 32 rows get zero weights.
    w2k = np.ascontiguousarray(
        w2.transpose(2, 1, 3, 0).reshape(96, 3, 64), np.float32)
    w2lo = np.zeros((128, 3, 64), np.float32)
    w2hi = np.zeros((128, 3, 64), np.float32)
    w2lo[0:96] = w2k
    w2hi[32:128] = w2k
    w2cat = np.concatenate([w2lo, w2hi], axis=2)  # [128, 3, 128]
    b2t = np.ascontiguousarray(np.tile(b2, 2)[:, None], np.float32)     # [128,1]

    # fc1: fc1wp[par*64+c, t2*12+xo, j] = fc1_w[j, c*144 + (2*t2+par)*12 + xo]
    fw = fc1_w.reshape(128, 64, 12, 12).transpose(1, 2, 3, 0)  # [c, yp, xo, j]
    fc1wp = np.ascontiguousarray(
        np.stack([fw[:, 0::2], fw[:, 1::2]]).reshape(128, 72, 128), np.float32)
    fc1bt = np.ascontiguousarray(fc1_b[:, None], np.float32)
    fc2wt = np.ascontiguousarray(fc2_w.T, np.float32)                   # [128,10]
    fc2bt = np.ascontiguousarray(fc2_b[:, None], np.float32)            # [10,1]

    weights = dict(w1p=W1p, b1t=b1t, w2cat=w2cat, b2t=b2t,
                   fc1wp=fc1wp, fc1bt=fc1bt, fc2wt=fc2wt, fc2bt=fc2bt)
    if MM_DT == "bfloat16":
        import ml_dtypes
        bf = ml_dtypes.bfloat16
        x3 = x3.astype(bf)
        for k in ("w1p", "w2cat", "fc1wp", "fc2wt"):
            weights[k] = weights[k].astype(bf)
    return x3, weights


def _build(bc):
    from concourse import bacc, tile
    import concourse.mybir as mybir

    from concourse.alu_op_type import AluOpType as Alu

    f32 = mybir.dt.float32
    mmdt = getattr(mybir.dt, MM_DT)
    Relu = mybir.ActivationFunctionType.Relu

    nc = bacc.Bacc("TRN2", target_bir_lowering=False, debug=False)

    x3_d = nc.dram_tensor("x3", [84, bc, 28], mmdt, kind="ExternalInput")
    w1_d = nc.dram_tensor("w1p", [84, 12, 128], mmdt, kind="ExternalInput")
    b1_d = nc.dram_tensor("b1t", [128, 1], f32, kind="ExternalInput")
    w2_d = nc.dram_tensor("w2cat", [128, 3, 128], mmdt, kind="ExternalInput")
    b2_d = nc.dram_tensor("b2t", [128, 1], f32, kind="ExternalInput")
    f1w_d = nc.dram_tensor("fc1wp", [128, 72, 128], mmdt, kind="ExternalInput")
    f1b_d = nc.dram_tensor("fc1bt", [128, 1], f32, kind="ExternalInput")
    f2w_d = nc.dram_tensor("fc2wt", [128, 10], mmdt, kind="ExternalInput")
    f2b_d = nc.dram_tensor("fc2bt", [10, 1], f32, kind="ExternalInput")
    out_d = nc.dram_tensor("out", [10, bc], f32, kind="ExternalOutput")

    n_sc = bc // 128            # fc1 superchunks of 128 images

    with tile.TileContext(nc) as tc:
        with (
            tc.tile_pool(name="wpool", bufs=1) as wp,
            tc.tile_pool(name="xpool", bufs=4) as xp,
            tc.tile_pool(name="h1pool", bufs=24) as h1p,
            tc.tile_pool(name="tmp", bufs=6) as tp,
            tc.tile_pool(name="pooled", bufs=1) as plp,
            tc.tile_pool(name="fc1h", bufs=1) as fhp,
            tc.tile_pool(name="outp", bufs=1) as op_,
            tc.tile_pool(name="ps1", bufs=3, space="PSUM") as ps1,
            tc.tile_pool(name="ps2", bufs=4, space="PSUM") as ps2,
            tc.tile_pool(name="psf", bufs=1, space="PSUM") as psf,
        ):
            w1_sb = wp.tile([84, 12, 128], mmdt)
            nc.sync.dma_start(w1_sb[:], w1_d[:])
            w2_sb = wp.tile([128, 3, 128], mmdt)
            nc.sync.dma_start(w2_sb[:], w2_d[:])
            f1w_sb = wp.tile([128, 72, 128], mmdt)
            nc.sync.dma_start(f1w_sb[:], f1w_d[:])
            f2w_sb = wp.tile([128, 10], mmdt)
            nc.sync.dma_start(f2w_sb[:], f2w_d[:])
            b1_sb = wp.tile([128, 1], f32)
            nc.sync.dma_start(b1_sb[:], b1_d[:])
            b2_sb = wp.tile([128, 1], f32)
            nc.sync.dma_start(b2_sb[:], b2_d[:])
            f1b_sb = wp.tile([128, 1], f32)
            nc.sync.dma_start(f1b_sb[:], f1b_d[:])
            f2b_sb = wp.tile([10, 1], f32)
            nc.sync.dma_start(f2b_sb[:], f2b_d[:])

            fc1h = fhp.tile([128, n_sc, 128], mmdt)

            for sc in range(n_sc):
                pooled = plp.tile([128, 128, 6, 12], mmdt, tag="pooled")
                for pci in range(4):
                    pc = sc * 4 + pci
                    h1t = [h1p.tile([128, 32, 26], mmdt, tag="h1",
                                    name=f"h1_{pc}_{j}")
                           for j in range(12)]
                    for half in range(2):
                        i0 = pc * 32 + half * 16
                        hs = slice(half * 16, half * 16 + 16)
                        x3t = xp.tile([84, 16, 28], mmdt, tag="x3")
                        nc.sync.dma_start(x3t[:], x3_d[:, i0:i0 + 16, :])
                        for g in range(12):
                            p = ps1.tile([128, 16, 26], f32, tag="c1ps",
                                         name=f"c1_{pc}_{half}_{g}")
                            nc.tensor.matmul(p[:], w1_sb[:, g, :],
                                             x3t[:, :, 0:26],
                                             start=True, stop=True)
                            # psum group g == h1 storage group g (yi = 2g+yl)
                            if g % 3 == 2:
                                nc.vector.tensor_scalar(
                                    h1t[g][:, hs, :], p[:], b1_sb[:], 0.0,
                                    op0=Alu.add, op1=Alu.max)
                            else:
                                nc.scalar.activation(h1t[g][:, hs, :], p[:],
                                                     Relu, bias=b1_sb[:])
                    for t in range(12):
                        par = (t % 2) * 64
                        t2 = t // 2
                        i0 = pci * 32
                        h2t = tp.tile([128, 32, 12, 2], mmdt, tag="h2",
                                      name=f"h2_{pc}_{t}")
                        for half in range(2):
                            hsl = slice(half * 16, half * 16 + 16)
                            pt = ps2.tile([128, 16, 12, 2], f32, tag="c2ps",
                                          name=f"c2_{pc}_{t}_{half}")
                            for dx in range(3):
                                nc.tensor.matmul(
                                    pt[:], w2_sb[:, dx, :],
                                    h1t[t][:, hsl, dx:dx + 24],
                                    start=(dx == 0), stop=(dx == 2))
                            # drain psum with fused relu+bias
                            if (t + half) % 3 == 0:
                                nc.vector.tensor_scalar(
                                    h2t[:, hsl, :, :], pt[:], b2_sb[:], 0.0,
                                    op0=Alu.add, op1=Alu.max)
                            else:
                                nc.scalar.activation(h2t[:, hsl, :, :], pt[:],
                                                     Relu, bias=b2_sb[:])
                        # pool: xo-pair maxes per parity half (same-base
                        # inputs), then cross-parity max into pooled
                        me = tp.tile([64, 32, 12], mmdt, tag="me",
                                     name=f"me_{pc}_{t}")
                        mo = tp.tile([64, 32, 12], mmdt, tag="mo",
                                     name=f"mo_{pc}_{t}")
                        nc.vector.tensor_max(me[:], h2t[0:64, :, :, 0],
                                             h2t[0:64, :, :, 1])
                        nc.vector.tensor_max(mo[:], h2t[64:128, :, :, 0],
                                             h2t[64:128, :, :, 1])
                        nc.vector.tensor_max(
                            pooled[par:par + 64, i0:i0 + 32, t2, :],
                            me[:], mo[:])
                fps = psf.tile([128, 128], f32, tag="fps", padded_shape=[128, 512])
                for r in range(72):
                    nc.tensor.matmul(fps[:], f1w_sb[:, r, :],
                                     pooled[:, :, r // 12, r % 12],
                                     start=(r == 0), stop=(r == 71))
                nc.scalar.activation(fc1h[:, sc, :], fps[:], Relu,
                                     bias=f1b_sb[:])

            f2ps = psf.tile([10, bc], f32, tag="fps")
            nc.tensor.matmul(f2ps[:], f2w_sb[:], fc1h[:],
                             start=True, stop=True)
            outsb = op_.tile([10, bc], f32)
            nc.vector.tensor_scalar_add(outsb[:], f2ps[:], f2b_sb[:])
            nc.sync.dma_start(out_d[:], outsb[:])

    nc.compile()
    return nc


def _get_nc(bc):
    key = (bc, MM_DT)
    if key not in _BUILT:
        _BUILT[key] = _build(bc)
    return _BUILT[key]


def run_cores(x, w1, b1, w2, b2, fc1_w, fc1_b, fc2_w, fc2_b,
              trace=False, ncores=NCORES):
    from concourse.bass_utils import run_bass_kernel_spmd

    B = x.shape[0]
    bc = B // ncores
    nc = _get_nc(bc)
    x3, weights = _host_prep(x, w1, b1, w2, b2, fc1_w, fc1_b,
                             fc2_w, fc2_b)
    in_maps = []
    for c in range(ncores):
        m = dict(weights)
        m["x3"] = np.ascontiguousarray(x3[:, c * bc:(c + 1) * bc, :])
        in_maps.append(m)
    res = run_bass_kernel_spmd(nc, in_maps, core_ids=list(range(ncores)),
                               trace=trace)
    out = np.empty((B, 10), np.float32)
    for c in range(ncores):
        out[c * bc:(c + 1) * bc] = res.results[c]["out"].T
    return out, res


def kernel(x, w1, b1, w2, b2, fc1_w, fc1_b, fc2_w, fc2_b):
    out, _ = run_cores(x, w1, b1, w2, b2, fc1_w, fc1_b, fc2_w, fc2_b)
    return out
